# revision 8
# baseline (speedup 1.0000x reference)
"""Trainium2 Bass kernel for nn_DeleteEdgeDecoder.

reference semantics (per batch b):
    feats[e] = [emb[i_e] | emb[j_e] | dist_e]          (513)
    h        = relu(feats @ W1 + b1)                   (E, 512)
    logits   = (h @ W2 + b2)[:, 0]  masked(-inf) + delete_bias

Sharding: pure data parallel, batch dim 128 -> 8 cores x 16.

Device program (per 128-edge block, edge-major psum [e=128, h=512]):
  - L1 on PE in fp8e4m3 with MatmulPerfMode.DoubleRow (2 K-tiles per
    instruction at 0.5 cyc/row = 4x fp16 throughput). Straight e4m3 fails
    the 2e-2 gate (3.5e-2), so the GEMM is error-corrected:
        W scaled by 16 (embeds W1's sigma~0.044 in e4m3's normal range;
        undone for free by relu's positive-scale commutativity),
        Wq  = q8(16 W1), eps = 16 W1 - Wq
        f8  = q8(feats), d
        z*16 = f8@Wq + q8(4(f-f8))@(Wq/4) + (f8/4)@q8(4 eps)
               + DR K=1-pair [dist;1]x[16wd;16b1]
    7 DR matmuls/block ~= 750ns; measured end-to-end rel err ~1e-3.
  - ACT: hr = relu(psum) -> fp16 (= 16*relu(z)).
  - DVE: one fused scalar_tensor_tensor per block:
        scratch = (hr * 1.0) * bcast(W2/16),  accum_out = row-sum
    i.e. logits column [128,1] directly; no separate mult+reduce passes.
  - Per batch: one [128,16] add of postbias (valid-mask -inf + b2 +
    delete_bias) and one f32 DMA out; host reassembles e = c*128+p.
"""

import os
from contextlib import ExitStack

import numpy as np
import ml_dtypes
import concourse.bass as bass
import concourse.bacc as bacc
import concourse.mybir as mybir
import concourse.tile as tile
from concourse.bass_utils import run_bass_kernel_spmd

B, N, D, E = 128, 2000, 256, 2000
NCORES = 8
BL = B // NCORES          # batches per core
EP = 2048                 # edges padded
H = 512
KT = 4                    # k-tiles over [emb_i|emb_j] (4 x 128)
EB = EP // 128            # 16 edge-blocks per batch

F16 = mybir.dt.float16
F32 = mybir.dt.float32
F8 = mybir.dt.float8e4
NP8 = ml_dtypes.float8_e4m3
DR = mybir.MatmulPerfMode.DoubleRow

_CACHE: dict = {}


def q8(x):
    return np.asarray(x, np.float32).astype(NP8)


def _build_nc(bl: int = BL):
    nc = bacc.Bacc(
        "TRN2", target_bir_lowering=False, debug=False, num_devices=NCORES
    )
    f8 = nc.dram_tensor("f8", [bl, 128, KT * EP], F8, kind="ExternalInput")
    d8 = nc.dram_tensor("d8", [bl, 128, KT * EP], F8, kind="ExternalInput")
    f84 = nc.dram_tensor("f84", [bl, 128, KT * EP], F8, kind="ExternalInput")
    disto = nc.dram_tensor("disto", [bl, 2, EP], F8, kind="ExternalInput")
    pb = nc.dram_tensor("pb", [bl, 128, EB], F32, kind="ExternalInput")
    w1q = nc.dram_tensor("w1q", [128, KT * H], F8, kind="ExternalInput")
    wq4 = nc.dram_tensor("wq4", [128, KT * H], F8, kind="ExternalInput")
    eps8 = nc.dram_tensor("eps8", [128, KT * H], F8, kind="ExternalInput")
    wdb1 = nc.dram_tensor("wdb1", [2, H], F8, kind="ExternalInput")
    w2b16 = nc.dram_tensor("w2b16", [128, H], F16, kind="ExternalInput")
    out = nc.dram_tensor("out", [bl, 128, EB], F32, kind="ExternalOutput")

    with tile.TileContext(nc) as tc, ExitStack() as ctx:
        const = ctx.enter_context(tc.tile_pool(name="const", bufs=1))
        ftp = ctx.enter_context(tc.tile_pool(name="ft", bufs=2))
        dtp = ctx.enter_context(tc.tile_pool(name="dt", bufs=2))
        f4p = ctx.enter_context(tc.tile_pool(name="f4", bufs=2))
        dop = ctx.enter_context(tc.tile_pool(name="do", bufs=2))
        pbp = ctx.enter_context(tc.tile_pool(name="pb", bufs=2))
        hrp = ctx.enter_context(tc.tile_pool(name="hr", bufs=4))
        scp = ctx.enter_context(tc.tile_pool(name="sc", bufs=3))
        lap = ctx.enter_context(tc.tile_pool(name="la", bufs=2))
        lfp = ctx.enter_context(tc.tile_pool(name="lf", bufs=2))
        psp = ctx.enter_context(tc.tile_pool(name="ps", bufs=6, space="PSUM"))
        wpsp = ctx.enter_context(tc.tile_pool(name="wps", bufs=1, space="PSUM"))

        # PE clock pre-warm: the Tensor engine ramps to full clock only
        # after ~3us of continuous execution; a chain of tiny matmuls keeps
        # PE busy through the initial DMA fill.
        warm = const.tile([128, 64], F16)
        nc.vector.memset(warm[:], 0.0)
        wps = wpsp.tile([64, 64], F32, tag="warm")
        for i in range(126):
            nc.tensor.matmul(wps[:], warm[:, :], warm[:, :],
                             start=True, stop=True)

        # batch 0's big fp8 feats chunks first so they overlap the const
        # uploads on the other queues.
        ft0 = ftp.tile([128, KT, EP], F8, tag="ft")
        nc.sync.dma_start(ft0[:], f8.ap()[0])
        f840 = f4p.tile([128, KT, EP], F8, tag="f4")
        nc.sync.dma_start(f840[:], f84.ap()[0])
        dt0 = dtp.tile([128, KT, EP], F8, tag="dt")
        nc.gpsimd.dma_start(dt0[:], d8.ap()[0])

        w1q_sb = const.tile([128, KT, H], F8)
        nc.scalar.dma_start(w1q_sb[:], w1q.ap())
        wq4_sb = const.tile([128, KT, H], F8)
        nc.scalar.dma_start(wq4_sb[:], wq4.ap())
        eps8_sb = const.tile([128, KT, H], F8)
        nc.scalar.dma_start(eps8_sb[:], eps8.ap())
        wdb1_sb = const.tile([1, 2, H], F8)
        nc.scalar.dma_start(wdb1_sb[:, 0, :], wdb1.ap()[0:1, :])
        nc.scalar.dma_start(wdb1_sb[:, 1, :], wdb1.ap()[1:2, :])
        w2b16_sb = const.tile([128, H], F16)
        nc.scalar.dma_start(w2b16_sb[:], w2b16.ap())

        for b in range(bl):
            if b == 0:
                ft, dt, ff = ft0, dt0, f840
            else:
                ft = ftp.tile([128, KT, EP], F8, tag="ft")
                nc.sync.dma_start(ft[:], f8.ap()[b])
                ff = f4p.tile([128, KT, EP], F8, tag="f4")
                nc.sync.dma_start(ff[:], f84.ap()[b])
                dt = dtp.tile([128, KT, EP], F8, tag="dt")
                nc.gpsimd.dma_start(dt[:], d8.ap()[b])
            do = dop.tile([1, 2, EP], F8, tag="do")
            nc.gpsimd.dma_start(do[:, 0, :], disto.ap()[b][0:1, :])
            nc.gpsimd.dma_start(do[:, 1, :], disto.ap()[b][1:2, :])
            pbt = pbp.tile([128, EB], F32, tag="pb")
            nc.gpsimd.dma_start(pbt[:], pb.ap()[b])

            la = lap.tile([128, EB], F32, tag="la")
            for eb in range(EB):
                es = slice(eb * 128, (eb + 1) * 128)
                ph = psp.tile([128, H], F32, tag="ps")
                nc.tensor.matmul(ph[:], ft[:, 0:2, es], w1q_sb[:, 0:2, :],
                                 start=True, stop=False, perf_mode=DR)
                nc.tensor.matmul(ph[:], ft[:, 2:4, es], w1q_sb[:, 2:4, :],
                                 start=False, stop=False, perf_mode=DR)
                nc.tensor.matmul(ph[:], dt[:, 0:2, es], wq4_sb[:, 0:2, :],
                                 start=False, stop=False, perf_mode=DR)
                nc.tensor.matmul(ph[:], dt[:, 2:4, es], wq4_sb[:, 2:4, :],
                                 start=False, stop=False, perf_mode=DR)
                nc.tensor.matmul(ph[:], ff[:, 0:2, es], eps8_sb[:, 0:2, :],
                                 start=False, stop=False, perf_mode=DR)
                nc.tensor.matmul(ph[:], ff[:, 2:4, es], eps8_sb[:, 2:4, :],
                                 start=False, stop=False, perf_mode=DR)
                # dist*16wd + 16*b1 as a K=1-pair DoubleRow rank-2 update
                nc.tensor.matmul(ph[:], do[:, :, es], wdb1_sb[:],
                                 start=False, stop=True, perf_mode=DR)
                hr = hrp.tile([128, H], F16, tag="hr")
                nc.scalar.activation(
                    hr[:], ph[:], mybir.ActivationFunctionType.Relu
                )
                # logits col = sum_h hr * (w2/16); scratch out is discarded
                sc = scp.tile([128, H], F16, tag="sc")
                nc.vector.scalar_tensor_tensor(
                    sc[:], hr[:], 1.0, w2b16_sb[:],
                    mybir.AluOpType.mult, mybir.AluOpType.mult,
                    accum_out=la[:, eb : eb + 1],
                )
            laf = lfp.tile([128, EB], F32, tag="lf")
            nc.vector.tensor_tensor(
                laf[:], la[:], pbt[:], mybir.AluOpType.add
            )
            nc.scalar.dma_start(out.ap()[b], laf[:])

    nc.compile()
    return nc


def _prep_weights(W1, b1, W2, b2, delete_bias):
    """Quantized weight-side tensors (shared across cores)."""
    W1 = np.asarray(W1, np.float32)
    Wemb = 16.0 * W1[: 2 * D]                      # (512, H)
    Wq = q8(Wemb)
    eps = Wemb - Wq.astype(np.float32)
    w1q = np.ascontiguousarray(
        Wq.astype(np.float32).reshape(KT, 128, H).transpose(1, 0, 2)
        .reshape(128, KT * H)
    )
    wq4 = q8(w1q / 4.0)
    w1q = q8(w1q)
    eps8 = q8(
        (4.0 * eps).reshape(KT, 128, H).transpose(1, 0, 2).reshape(128, KT * H)
    )
    wdb1 = q8(np.stack([16.0 * W1[2 * D], 16.0 * np.asarray(b1)]))  # (2, H)
    w2b16 = np.ascontiguousarray(
        np.broadcast_to(
            (np.asarray(W2, np.float32)[:, 0] / 16.0).astype(np.float16),
            (128, H),
        )
    )
    return {"w1q": w1q, "wq4": wq4, "eps8": eps8, "wdb1": wdb1,
            "w2b16": w2b16}


def _prep_core_inputs(core, node_embeddings, locs, edge_list, delete_bias,
                      W1, b1, W2, b2, bl: int = BL, weights=None):
    """Per-core input map (host gather + fp8 hi/lo marshalling)."""
    b0 = core * bl
    emb = np.asarray(node_embeddings[b0 : b0 + bl], np.float32)  # (bl, N, D)

    el = edge_list[b0 : b0 + bl]  # (bl, E, 2) int
    iclip = np.maximum(el[..., 0], 0).astype(np.int64)
    jclip = np.maximum(el[..., 1], 0).astype(np.int64)
    ipad = np.zeros((bl, EP), dtype=np.int64)
    ipad[:, :E] = iclip
    jpad = np.zeros((bl, EP), dtype=np.int64)
    jpad[:, :E] = jclip

    bidx = np.arange(bl)[:, None]
    gi = emb[bidx, ipad]  # (bl, EP, D)
    gj = emb[bidx, jpad]
    fti = gi.reshape(bl, EP, 2, 128).transpose(0, 3, 2, 1)  # (bl,128,2,EP)
    ftj = gj.reshape(bl, EP, 2, 128).transpose(0, 3, 2, 1)
    featsT = np.concatenate([fti, ftj], axis=2).reshape(bl, 128, KT * EP)
    f8 = q8(featsT)
    d8 = q8(4.0 * (featsT - f8.astype(np.float32)))
    f84 = q8(f8.astype(np.float32) / 4.0)

    lc = np.asarray(locs[b0 : b0 + bl], np.float32)
    dvec = lc[bidx, iclip] - lc[bidx, jclip]
    dist = np.sqrt((dvec * dvec).sum(-1))  # (bl, E)
    disto = np.zeros((bl, 2, EP), dtype=NP8)
    disto[:, 0, :E] = q8(dist)
    disto[:, 1, :] = 1.0

    valid = (el[..., 0] >= 0) & (el[..., 1] >= 0)
    pbfull = np.zeros((bl, EP), dtype=np.float32)
    pbfull[:, :E] = (
        np.where(valid, 0.0, -np.inf)
        + float(np.asarray(b2).reshape(-1)[0])
        + float(delete_bias)
    )
    pb = np.ascontiguousarray(
        pbfull.reshape(bl, EB, 128).transpose(0, 2, 1)
    )  # pb[b, p, c] = pbfull[b, c*128+p]

    if weights is None:
        weights = _prep_weights(W1, b1, W2, b2, delete_bias)
    return {
        "f8": np.ascontiguousarray(f8),
        "d8": np.ascontiguousarray(d8),
        "f84": np.ascontiguousarray(f84),
        "disto": disto,
        "pb": pb,
        **weights,
    }


def kernel(node_embeddings, locs, edge_list, delete_bias, W1, b1, W2, b2):
    node_embeddings = np.asarray(node_embeddings, dtype=np.float32)
    locs = np.asarray(locs, dtype=np.float32)
    edge_list = np.asarray(edge_list)
    W1 = np.asarray(W1, dtype=np.float32)
    b1 = np.asarray(b1, dtype=np.float32)
    W2 = np.asarray(W2, dtype=np.float32)
    b2 = np.asarray(b2, dtype=np.float32)

    if "nc" not in _CACHE:
        _CACHE["nc"] = _build_nc()
    nc = _CACHE["nc"]

    weights = _prep_weights(W1, b1, W2, b2, delete_bias)
    in_maps = [
        _prep_core_inputs(c, node_embeddings, locs, edge_list, delete_bias,
                          W1, b1, W2, b2, weights=weights)
        for c in range(NCORES)
    ]
    trace = os.environ.get("BASS_KERNEL_TRACE", "0") == "1"
    res = run_bass_kernel_spmd(nc, in_maps, list(range(NCORES)), trace=trace)
    _CACHE["last_result"] = res

    outs = []
    for c in range(NCORES):
        o = np.asarray(res.results[c]["out"], dtype=np.float32)  # (bl,128,EB)
        o = o.transpose(0, 2, 1).reshape(BL, EP)  # e = c*128 + p
        outs.append(o[:, :E])
    return np.concatenate(outs, axis=0)
